# revision 11
# baseline (speedup 1.0000x reference)
import os
import numpy as np

EPS = 1e-6
BLOCK = 64
B, H, L, D, F = 2, 16, 4096, 64, 64
F2 = 2 * F
NBLK = L // BLOCK          # 64 blocks per (b,h)
LANES = 4                  # (b,h) pairs per core: 2 heads x 2 batches
GROUPS = 4                 # groups of 16 blocks
GBLK = 16                  # blocks per group
GPAIR = 8                  # block-pairs per group
SCALING = float(D) ** -0.5
NCORES = 8

# ---------------------------------------------------------------------------
# numpy fallback (used only if the device path raises)
# ---------------------------------------------------------------------------

def _softmax_lastaxis(u):
    m = u.max(axis=-1, keepdims=True)
    e = np.exp(u - m)
    return e / e.sum(axis=-1, keepdims=True)


def _kernel_numpy(query_states, key_states, value_states, hedgehog_weights, alpha):
    q = np.asarray(query_states, dtype=np.float32)
    k = np.asarray(key_states, dtype=np.float32)
    v = np.asarray(value_states, dtype=np.float32)
    W = np.asarray(hedgehog_weights, dtype=np.float32)
    a = np.asarray(alpha, dtype=np.float32)
    Bq, Hq, Lq, Dq = q.shape
    S = BLOCK
    N = Lq // S
    qb = q.reshape(Bq, Hq, N, S, Dq)
    kb = k.reshape(Bq, Hq, N, S, Dq)
    vb = v.reshape(Bq, Hq, N, S, Dq)
    uq = np.einsum("bhnsd,hdf->bhnsf", qb, W, optimize=True)
    uk = np.einsum("bhnsd,hdf->bhnsf", kb, W, optimize=True)
    phi_q = np.concatenate([_softmax_lastaxis(uq), _softmax_lastaxis(-uq)], axis=-1)
    phi_k = np.concatenate([_softmax_lastaxis(uk), _softmax_lastaxis(-uk)], axis=-1)
    w = 1.0 / (1.0 + np.exp(-a))
    S_state = np.zeros((Bq, Hq, 2 * F, Dq), np.float32)
    Z_state = np.zeros((Bq, Hq, 2 * F), np.float32)
    outs = np.empty((Bq, Hq, N, S, Dq), np.float32)
    for n in range(N):
        qn, kn, vn = qb[:, :, n], kb[:, :, n], vb[:, :, n]
        pqn, pkn = phi_q[:, :, n], phi_k[:, :, n]
        lin_num = np.einsum("bhsf,bhfd->bhsd", pqn, S_state, optimize=True)
        lin_den = np.maximum(
            np.einsum("bhsf,bhf->bhs", pqn, Z_state, optimize=True)[..., None], EPS
        )
        scores = np.einsum("bhsd,bhtd->bhst", qn, kn, optimize=True) * SCALING
        aexp = np.exp(scores - scores.max(axis=-1, keepdims=True))
        sm_num = np.einsum("bhst,bhtd->bhsd", aexp, vn, optimize=True)
        sm_den = np.maximum(aexp.sum(axis=-1, keepdims=True), EPS)
        outs[:, :, n] = (w * sm_num + lin_num) / np.maximum(w * sm_den + lin_den, EPS)
        S_state = S_state + np.einsum("bhsf,bhsd->bhfd", pkn, vn, optimize=True)
        Z_state = Z_state + pkn.sum(axis=-2)
    return outs.reshape(Bq, Hq, Lq, Dq)


# ---------------------------------------------------------------------------
# device kernel
# ---------------------------------------------------------------------------

_BASS_CACHE = {}


def _build_bass():
    if "nc" in _BASS_CACHE:
        return _BASS_CACHE["nc"]
    import concourse.bass as bass
    import concourse.tile as tile
    from concourse import mybir
    from contextlib import ExitStack

    fp32 = mybir.dt.float32
    bf16 = mybir.dt.bfloat16
    AF = mybir.ActivationFunctionType
    OP = mybir.AluOpType

    nc = bass.Bass("TRN2")
    qt_d = nc.dram_tensor("qt", (LANES, GROUPS, 64, GBLK * 64), bf16, kind="ExternalInput")
    kt_d = nc.dram_tensor("kt", (LANES, GROUPS, 64, GBLK * 64), bf16, kind="ExternalInput")
    v1_d = nc.dram_tensor("v1", (LANES, GROUPS, 128, GPAIR, 65), bf16, kind="ExternalInput")
    w_d = nc.dram_tensor("wmat", (64, 2, 64), bf16, kind="ExternalInput")
    lw_d = nc.dram_tensor("logw", (128, LANES), fp32, kind="ExternalInput")
    out_d = nc.dram_tensor("out", (LANES, GROUPS, 4, 128, 2, 64), fp32, kind="ExternalOutput")

    with tile.TileContext(nc) as tc, ExitStack() as ctx:
        singles = ctx.enter_context(tc.tile_pool(name="singles", bufs=1))
        qk_pool = ctx.enter_context(tc.tile_pool(name="qk", bufs=4))
        v_pool = ctx.enter_context(tc.tile_pool(name="vp", bufs=5))
        u_pool = ctx.enter_context(tc.tile_pool(name="up", bufs=2, space="PSUM"))
        e_pool = ctx.enter_context(tc.tile_pool(name="ep", bufs=2))
        r_pool = ctx.enter_context(tc.tile_pool(name="rp", bufs=2))
        phi_pool = ctx.enter_context(tc.tile_pool(name="phip", bufs=5))
        phiT_pool = ctx.enter_context(tc.tile_pool(name="phitp", bufs=5))
        sc_pool = ctx.enter_context(tc.tile_pool(name="scp", bufs=2, space="PSUM"))
        sct_pool = ctx.enter_context(tc.tile_pool(name="sctp", bufs=2))
        a_pool = ctx.enter_context(tc.tile_pool(name="ap", bufs=2))
        aT_pool = ctx.enter_context(tc.tile_pool(name="atp", bufs=5))
        st_pool = ctx.enter_context(tc.tile_pool(name="stp", bufs=1, space="PSUM"))
        ssb_pool = ctx.enter_context(tc.tile_pool(name="ssbp", bufs=2))
        num_pool = ctx.enter_context(tc.tile_pool(name="nump", bufs=2, space="PSUM"))
        den_pool = ctx.enter_context(tc.tile_pool(name="denp", bufs=3))
        o_pool = ctx.enter_context(tc.tile_pool(name="op", bufs=3))

        # constants
        w_sb = singles.tile([64, 2, 64], bf16)
        nc.sync.dma_start(out=w_sb, in_=w_d[:, :, :])
        lw_sb = singles.tile([128, LANES], fp32)
        nc.sync.dma_start(out=lw_sb, in_=lw_d[:, :])

        # persistent state accumulator: (128, lane, 65) fp32 in PSUM
        state = st_pool.tile([128, LANES, 65], fp32)

        for g in range(GROUPS):
            # ---------------- feature + scores phase (per lane) -----------
            lane_tiles = []
            for lane in range(LANES):
                hh = lane // 2
                qt_t = qk_pool.tile([64, GBLK * 64], bf16, tag="qt")
                kt_t = qk_pool.tile([64, GBLK * 64], bf16, tag="kt")
                v1_t = v_pool.tile([128, GPAIR, 65], bf16, tag="v1")
                nc.sync.dma_start(out=qt_t, in_=qt_d[lane, g, :, :])
                nc.sync.dma_start(out=kt_t, in_=kt_d[lane, g, :, :])
                nc.sync.dma_start(out=v1_t, in_=v1_d[lane, g, :, :, :])

                phis = {}
                for which, src_t in (("q", qt_t), ("k", kt_t)):
                    u_t = u_pool.tile([128, GPAIR, 64], fp32, tag="u")
                    for j in range(GPAIR):
                        nc.tensor.matmul(
                            u_t[:, j, :],
                            src_t[:, 128 * j:128 * (j + 1)],
                            w_sb[:, hh, :],
                            start=True, stop=True,
                        )
                    ep_t = e_pool.tile([128, GPAIR, 64], bf16, tag="eplus")
                    em_t = e_pool.tile([128, GPAIR, 64], bf16, tag="eminus")
                    nc.scalar.activation(ep_t, u_t, AF.Exp, scale=1.0)
                    nc.scalar.activation(em_t, u_t, AF.Exp, scale=-1.0)
                    r_t = r_pool.tile([128, 2, GPAIR, 1], fp32, tag="r")
                    nc.vector.reduce_sum(r_t[:, 0], ep_t, axis=mybir.AxisListType.X)
                    nc.vector.reduce_sum(r_t[:, 1], em_t, axis=mybir.AxisListType.X)
                    ri_t = r_pool.tile([128, 2, GPAIR, 1], fp32, tag="ri")
                    nc.vector.reciprocal(ri_t, r_t)
                    phi_t = phi_pool.tile([128, GPAIR, 2, 64], bf16, tag="phi" + which)
                    nc.gpsimd.tensor_tensor(
                        out=phi_t[:, :, 0, :], in0=ep_t,
                        in1=ri_t[:, 0].to_broadcast((128, GPAIR, 64)),
                        op=OP.mult)
                    nc.gpsimd.tensor_tensor(
                        out=phi_t[:, :, 1, :], in0=em_t,
                        in1=ri_t[:, 1].to_broadcast((128, GPAIR, 64)),
                        op=OP.mult)
                    phis[which] = phi_t

                # transpose phi_q pairs -> pqT quads (f2 on partitions)
                pqT_t = phiT_pool.tile([128, GPAIR, 128], bf16, tag="pqT")
                for j in range(GPAIR):
                    nc.sync.dma_start_transpose(pqT_t[:, j, :], phis["q"][:, j, :, :])

                # scores for 16 blocks: block n16 -> (phalf=(n16%4)//2, col=2*(n16//4)+(n16%2))
                sc_t = sc_pool.tile([128, GPAIR, 64], fp32, tag="sc")
                for n16 in range(GBLK):
                    ph = (n16 % 4) // 2
                    cidx = 2 * (n16 // 4) + (n16 % 2)
                    nc.tensor.matmul(
                        sc_t[64 * ph:64 * (ph + 1), cidx, :],
                        qt_t[:, 64 * n16:64 * (n16 + 1)],
                        kt_t[:, 64 * n16:64 * (n16 + 1)],
                        start=True, stop=True,
                        tile_position=(0, 64 * ph),
                    )
                mx_t = r_pool.tile([128, GPAIR, 1], fp32, tag="mx")
                nc.vector.reduce_max(mx_t, sc_t, axis=mybir.AxisListType.X)
                sct_t = sct_pool.tile([128, GPAIR, 64], fp32, tag="sct")
                nc.vector.tensor_tensor(
                    out=sct_t, in0=sc_t,
                    in1=mx_t.to_broadcast((128, GPAIR, 64)), op=OP.subtract)
                a_t = a_pool.tile([128, GPAIR, 64], bf16, tag="a")
                nc.scalar.activation(a_t, sct_t, AF.Exp,
                                     bias=lw_sb[:, lane:lane + 1], scale=SCALING)
                aT_t = aT_pool.tile([128, 4, 128], bf16, tag="aT")
                for c in range(4):
                    nc.sync.dma_start_transpose(aT_t[:, c, :], a_t[:, 2 * c:2 * c + 2, :])

                lane_tiles.append(dict(
                    v1=v1_t, pq=phis["q"], pk=phis["k"], pqT=pqT_t, aT=aT_t))

            # ---------------- scan phase: 16 blocks, 4 lanes lockstep -----
            for sg in range(4):
                nums = [num_pool.tile([128, 2, 2, 65], fp32, tag="num", name=f"num{_i}") for _i in range(2)]
                for n2 in range(4):
                    n16 = 4 * sg + n2
                    nglob = GBLK * g + n16
                    ph = (n16 % 4) // 2
                    base = 64 * (n16 % 2)
                    j = n16 // 2
                    cg = n16 % 2
                    if nglob > 0:
                        ssb_t = ssb_pool.tile([128, LANES, 65], bf16, tag="ssb")
                        nc.vector.tensor_copy(out=ssb_t, in_=state)
                    for lane in range(LANES):
                        lt = lane_tiles[lane]
                        num_t = nums[lane // 2]
                        out_ap = num_t[64 * ph:64 * (ph + 1), lane % 2, cg, :]
                        # softmax branch (+ w folded into a); den rides in col 64
                        nc.tensor.matmul(
                            out_ap,
                            lt["aT"][base:base + 64, n2, 64 * ph:64 * (ph + 1)],
                            lt["v1"][base:base + 64, j, :],
                            start=True, stop=(nglob == 0),
                            tile_position=(base, 64 * ph),
                        )
                        if nglob > 0:
                            nc.tensor.matmul(
                                out_ap,
                                lt["pqT"][:, j, base:base + 64],
                                ssb_t[:, lane, :],
                                start=False, stop=True,
                                tile_position=(0, 64 * ph),
                            )
                        # state update
                        nc.tensor.matmul(
                            state[:, lane, :],
                            lt["pk"][base:base + 64, j, :, :],
                            lt["v1"][base:base + 64, j, :],
                            start=(nglob == 0), stop=(nglob == NBLK - 1),
                            tile_position=(base, 0),
                            skip_group_check=True,
                        )
                # combine + store
                for lh in range(2):
                    den_t = den_pool.tile([128, 2, 2, 1], fp32, tag="den")
                    nc.vector.reciprocal(den_t, nums[lh][:, :, :, 64:65])
                    o_t = o_pool.tile([128, 2, 2, 64], fp32, tag="o")
                    nc.vector.tensor_tensor(
                        out=o_t, in0=nums[lh][:, :, :, 0:64],
                        in1=den_t.to_broadcast((128, 2, 2, 64)), op=OP.mult)
                    for ll in range(2):
                        lane = 2 * lh + ll
                        nc.sync.dma_start(out=out_d[lane, g, sg, :, :, :], in_=o_t[:, ll, :, :])

    _BASS_CACHE["nc"] = nc
    return nc


def _prep_inputs(q, k, v, W, alpha):
    """Build per-core device input maps (host-side sharding + layout)."""
    w_sig = 1.0 / (1.0 + np.exp(-alpha.astype(np.float64)))  # (1,H,1,1)
    logw = np.log(w_sig).astype(np.float32).reshape(H)

    in_maps = []
    for c in range(NCORES):
        heads = (2 * c, 2 * c + 1)
        lanes = [(b, heads[hh]) for hh in range(2) for b in range(B)]
        qt = np.empty((LANES, GROUPS, 64, GBLK * 64), np.float32)
        kt = np.empty((LANES, GROUPS, 64, GBLK * 64), np.float32)
        v1 = np.ones((LANES, GROUPS, 128, GPAIR, 65), np.float32)
        lw = np.empty((128, LANES), np.float32)
        for lane, (b, h) in enumerate(lanes):
            qT = q[b, h].T  # (64, 4096)
            kT = k[b, h].T
            qt[lane] = qT.reshape(64, GROUPS, GBLK * 64).transpose(1, 0, 2)
            kt[lane] = kT.reshape(64, GROUPS, GBLK * 64).transpose(1, 0, 2)
            vp = v[b, h].reshape(NBLK // 2, 128, 64)  # (pair, 128, 64)
            v1[lane, :, :, :, :64] = (
                vp.reshape(GROUPS, GPAIR, 128, 64).transpose(0, 2, 1, 3))
            lw[:, lane] = logw[h]
        wm = np.stack([W[heads[0]], W[heads[1]]], axis=1)  # (64, 2, 64)
        in_maps.append({
            "qt": qt,
            "kt": kt,
            "v1": v1,
            "wmat": wm.astype(np.float32),
            "logw": lw,
        })
    return in_maps


def _to_bf16(x):
    import ml_dtypes
    return np.asarray(x).astype(ml_dtypes.bfloat16)


def _install_ntff_hook():
    """Shim antenv.axon_hooks (absent in this image) so trace=True works."""
    import sys, types
    if "antenv.axon_hooks" in sys.modules:
        return
    mod = types.ModuleType("antenv.axon_hooks")
    mod._hook = None
    def set_axon_ntff_profile_hook(h):
        mod._hook = h
    def get_axon_ntff_profile_hook():
        return mod._hook
    mod.set_axon_ntff_profile_hook = set_axon_ntff_profile_hook
    mod.get_axon_ntff_profile_hook = get_axon_ntff_profile_hook
    sys.modules["antenv.axon_hooks"] = mod
    try:
        import antenv
        antenv.axon_hooks = mod
        from trn_agent_boot.trn_boot import _ntff_profile_via_ctypes
        set_axon_ntff_profile_hook(_ntff_profile_via_ctypes("/opt/axon/libaxon_pjrt.so"))
    except Exception:
        pass


def _run_device(query_states, key_states, value_states, hedgehog_weights, alpha):
    import concourse.bass_utils as bu
    from concourse.bass_utils import run_bass_kernel_spmd

    q = np.asarray(query_states, np.float32)
    k = np.asarray(key_states, np.float32)
    v = np.asarray(value_states, np.float32)
    W = np.asarray(hedgehog_weights, np.float32)
    al = np.asarray(alpha, np.float32)

    nc = _build_bass()
    raw_maps = _prep_inputs(q, k, v, W, al)
    in_maps = []
    for m in raw_maps:
        in_maps.append({
            "qt": _to_bf16(m["qt"]),
            "kt": _to_bf16(m["kt"]),
            "v1": _to_bf16(m["v1"]),
            "wmat": _to_bf16(m["wmat"]),
            "logw": m["logw"],
        })

    trace = os.environ.get("KERNEL_TRACE", "0") == "1"
    if trace:
        _install_ntff_hook()
        bu.upload_artifacts = lambda tmpdir: tmpdir
    res = run_bass_kernel_spmd(
        nc, in_maps, core_ids=list(range(NCORES)), trace=trace,
        tmpdir="/tmp/kern_trace" if trace else None)
    if res.exec_time_ns is not None:
        kernel.last_exec_ns = res.exec_time_ns

    out = np.empty((B, H, L, D), np.float32)
    for c in range(NCORES):
        heads = (2 * c, 2 * c + 1)
        lanes = [(b, heads[hh]) for hh in range(2) for b in range(B)]
        od = res.results[c]["out"]  # (LANES, GROUPS, 4, 128, 2, 64)
        # block n = 16g + 4sg + 2*(p//64) + cg ; s = p%64
        od = od.reshape(LANES, GROUPS, 4, 2, 64, 2, 64).transpose(0, 1, 2, 3, 5, 4, 6)
        od = od.reshape(LANES, L, D)
        for lane, (b, h) in enumerate(lanes):
            out[b, h] = od[lane]
    return out


def kernel(**inputs):
    try:
        return _run_device(**inputs)
    except Exception:
        import traceback
        traceback.print_exc()
        return _kernel_numpy(**inputs)
